# revision 1
# baseline (speedup 1.0000x reference)
"""KNN-conv kernel for Trainium2, data-parallel over batch on 8 NeuronCores.

Problem: for x (32, 128, 32, 32) and conv weight W (128, 128, 9):
  per batch: cosine-sim (1024x1024) over channels, diag -> +INF, top-9
  neighbors per token, gather neighbor features, contract with W.

Strategy per core (4 batches):
  - xn = x / ||x||_c  (true-fp32 PE matmuls for the similarity matrix; the
    top-k decision amplifies sim noise, so fp32r/bf16 are not usable here)
  - diag forced to -1e10 with a tiny accumulating matmul; rank-0 neighbor is
    always the token itself, so top-8 of the rest = one max + max_index pass
  - neighbor gather via dma_gather(transpose=True) from a host-prepared
    token-major [hi|lo] bf16 split (512B per token), giving channel-major
    tiles directly
  - conv = 2-pass bf16 matmuls (W*hi + W*lo) accumulated in fp32 PSUM
"""

import os

import numpy as np

B, C, N, K = 32, 128, 1024, 9
O = 128  # out channels
NCORES = 8
BPC = B // NCORES  # batches per core
NEG = -1.0e10

_prog_cache = {}
last_results = None  # BassKernelResults of the most recent run (for test.py)


def _build_program():
    import concourse.bacc as bacc
    import concourse.mybir as mybir
    from concourse.tile import TileContext

    f32 = mybir.dt.float32
    bf16 = mybir.dt.bfloat16
    u16 = mybir.dt.uint16
    i16 = mybir.dt.int16
    AF = mybir.ActivationFunctionType

    nc = bacc.Bacc()
    skip = set(os.environ.get("KNN_SKIP", "").split(","))

    x_h = nc.declare_dram_parameter("x", [BPC, C, N], f32, isOutput=False)
    xt_h = nc.declare_dram_parameter("xt", [BPC, N, 2 * C], bf16, isOutput=False)
    hilo_h = nc.declare_dram_parameter("hilo", [BPC, 2, C, N], bf16, isOutput=False)
    wt_h = nc.declare_dram_parameter("wt", [C, K * O], bf16, isOutput=False)
    ident_h = nc.declare_dram_parameter("ident", [128, 128], bf16, isOutput=False)
    negi_h = nc.declare_dram_parameter("negi", [128, 128], bf16, isOutput=False)
    ones128_h = nc.declare_dram_parameter("ones128", [C, 1], f32, isOutput=False)
    ones1_h = nc.declare_dram_parameter("ones1", [1, 128], f32, isOutput=False)
    out_h = nc.declare_dram_parameter("out", [BPC, O, N], f32, isOutput=True)

    idxd_h = nc.dram_tensor("idxd", [BPC, 8192], u16)
    rd_h = nc.dram_tensor("rd", [BPC, N], f32)

    with TileContext(nc) as tc:
        with (
            tc.tile_pool(name="consts", bufs=1) as consts,
            tc.tile_pool(name="xp", bufs=int(os.environ.get("KNN_XP","2"))) as xp,
            tc.tile_pool(name="sqp", bufs=2) as sqp,
            tc.tile_pool(name="xnp", bufs=int(os.environ.get("KNN_XP","2"))) as xnp,
            tc.tile_pool(name="scp", bufs=int(os.environ.get("KNN_SCP","3"))) as scp,
            tc.tile_pool(name="smallp", bufs=int(os.environ.get("KNN_SMALL","4"))) as smallp,
            tc.tile_pool(name="idxp", bufs=int(os.environ.get("KNN_IDXP","2"))) as idxp,
            tc.tile_pool(name="prp", bufs=10) as prp,
            tc.tile_pool(name="hlp", bufs=2) as hlp,
            tc.tile_pool(name="outp", bufs=2) as outp,
            tc.tile_pool(name="psb", bufs=3, space="PSUM") as psb,
            tc.tile_pool(name="pso", bufs=1, space="PSUM") as pso,
        ):
            wts = consts.tile([C, K * O], bf16, tag="wts")
            nc.sync.dma_start(out=wts[:], in_=wt_h[:])
            ident = consts.tile([128, 128], bf16, tag="ident")
            nc.sync.dma_start(out=ident[:], in_=ident_h[:])
            negi = consts.tile([128, 128], bf16, tag="negi")
            nc.sync.dma_start(out=negi[:], in_=negi_h[:])
            ones128 = consts.tile([C, 1], f32, tag="ones128")
            nc.sync.dma_start(out=ones128[:], in_=ones128_h[:])
            ones1 = consts.tile([1, 128], f32, tag="ones1")
            nc.sync.dma_start(out=ones1[:], in_=ones1_h[:])

            for b in range(BPC):
                # ---- load + normalize -------------------------------------
                X = xp.tile([C, N], f32, tag="x")
                nc.sync.dma_start(out=X[:], in_=x_h[b])

                SQ = sqp.tile([C, N], f32, tag="sq")
                nc.scalar.activation(SQ[:], X[:], AF.Square)

                # norm^2 transposed: n2[p, blk] = sum_c SQ[c, blk*128+p]
                n2 = psb.tile([128, 1024], f32, tag="ps_big")
                for blk in range(8):
                    nc.tensor.matmul(
                        n2[:, blk : blk + 1],
                        SQ[:, blk * 128 : (blk + 1) * 128],
                        ones128[:],
                        start=True,
                        stop=True,
                    )
                sq8 = smallp.tile([128, 8], f32, tag="sq8")
                nc.scalar.activation(sq8[:], n2[:, :8], AF.Sqrt)
                rA = smallp.tile([128, 8], f32, tag="rA")
                nc.vector.tensor_scalar_add(rA[:], sq8[:], 1e-8)
                rT = smallp.tile([128, 8], f32, tag="rT")
                nc.vector.reciprocal(rT[:], rA[:])
                # bounce (128, 8) -> token-ordered (1, 1024) via DRAM
                nc.sync.dma_start(
                    out=rd_h[b].rearrange("(blk p) -> p blk", p=128), in_=rT[:]
                )
                r1 = smallp.tile([1, N], f32, tag="r1")
                nc.sync.dma_start(
                    out=r1[:], in_=rd_h[b].rearrange("(one n) -> one n", one=1)
                )
                # broadcast r over partitions: R = ones1^T @ r1
                R = psb.tile([128, 1024], f32, tag="ps_big")
                nc.tensor.matmul(
                    R[:, :512], ones1[:], r1[:, :512], start=True, stop=True
                )
                nc.tensor.matmul(
                    R[:, 512:], ones1[:], r1[:, 512:], start=True, stop=True
                )
                XN = xnp.tile([C, N], f32, tag="xn")
                nc.vector.tensor_mul(XN[:], X[:], R[:])

                # ---- similarity + top-8 -----------------------------------
                IDX = idxp.tile([128, 64], u16, tag="idx")
                for c in range(8):
                    if "sim" in skip:
                        nc.vector.memset(IDX[:, c : 64 : 8], c)
                        continue
                    ps = psb.tile([128, 1024], f32, tag="ps_big")
                    lhs = XN[:, c * 128 : (c + 1) * 128]
                    nc.tensor.matmul(
                        ps[:, :512], lhs, XN[:, :512], start=True, stop=(c >= 4)
                    )
                    nc.tensor.matmul(
                        ps[:, 512:], lhs, XN[:, 512:], start=True, stop=(c < 4)
                    )
                    # diag block -> -1e10 (accumulate -1e10*I)
                    nc.tensor.matmul(
                        ps[:, c * 128 : c * 128 + 128],
                        ident[:],
                        negi[:],
                        start=False,
                        stop=True,
                    )
                    SC = scp.tile([128, N], f32, tag="sc")
                    nc.scalar.activation(SC[:], ps[:], AF.Copy)
                    if "topk" in skip:
                        nc.vector.memset(IDX[:, c : 64 : 8], c)
                        continue
                    V8 = smallp.tile([128, 8], f32, tag="v8")
                    nc.vector.max(V8[:], SC[:])
                    # rank-major layout IDX[p, j*8 + c] keeps the DMA shuffle
                    # below within the 3-dim AP limit
                    nc.vector.max_index(IDX[:, c : 64 : 8], V8[:], SC[:])

                # ---- index bounce to 16-wrapped gather layout -------------
                # IDX[p, 8j+c] -> IDXG[16g+q, 64j+8c+sl] with p = 16sl+q
                # (= idx of token 16s+q at wrap slot s, per gather contract)
                nc.sync.dma_start(out=idxd_h[b], in_=IDX[:])
                IDXG = idxp.tile([128, 512], u16, tag="idxg")
                for g in range(8):
                    nc.sync.dma_start(
                        out=IDXG[16 * g : 16 * g + 16, :].rearrange(
                            "q (kc sl) -> q kc sl", kc=64
                        ),
                        in_=idxd_h[b].rearrange(
                            "(sl q kc) -> q kc sl", sl=8, q=16, kc=64
                        ),
                    )

                # ---- gathers (channel-major hi/lo via transpose mode) -----
                prs = {}
                for k in range(1, 9):
                    PR = prp.tile([C, 2 * N], bf16, tag="pr")
                    nc.gpsimd.dma_gather(
                        out_ap=PR[:].rearrange("p (t n) -> p t n", t=2),
                        in_ap=xt_h[b],
                        idxs_ap=IDXG[:, (k - 1) * 64 : k * 64].bitcast(i16),
                        num_idxs=N,
                        num_idxs_reg=N,
                        elem_size=2 * C,
                        transpose=True,
                        # single_packet=True overflows the SWDGE packet limit in
                        # transpose mode and crashes the device; False works.
                        single_packet=False,
                    )
                    prs[k] = PR
                HILO = hlp.tile([C, 2 * N], bf16, tag="hilo")
                nc.sync.dma_start(
                    out=HILO[:].rearrange("c (t n) -> c t n", t=2),
                    in_=hilo_h[b].rearrange("t c n -> c t n"),
                )

                # ---- conv contraction (2-pass bf16) -----------------------
                # hi-only conv: prime quantized to bf16 adds ~0.3% output error
                # (no top-k amplification downstream), well inside budget
                PO = pso.tile([O, N], f32, tag="ps_out")
                for k in range(1 if "conv" in skip else 9):
                    w_k = wts[:, k * O : (k + 1) * O]
                    for h in range(2):
                        if k == 0:
                            src = HILO[:, h * 512 : (h + 1) * 512]
                        else:
                            src = prs[k][:, h * 512 : (h + 1) * 512]
                        nc.tensor.matmul(
                            PO[:, h * 512 : (h + 1) * 512],
                            w_k,
                            src,
                            start=(k == 0),
                            stop=(k == 8),
                        )
                OUT = outp.tile([O, N], f32, tag="out")
                nc.scalar.activation(OUT[:], PO[:], AF.Copy)
                nc.sync.dma_start(out=out_h[b], in_=OUT[:])

    nc.compile()
    return nc


def _get_program():
    if "nc" not in _prog_cache:
        _prog_cache["nc"] = _build_program()
    return _prog_cache["nc"]


def _host_prep(x, W):
    """Build per-core input maps from full inputs."""
    import ml_dtypes

    bf16 = ml_dtypes.bfloat16
    xf = np.ascontiguousarray(x.reshape(B, C, N).astype(np.float32, copy=False))
    hi = xf.astype(bf16)
    lo = (xf - hi.astype(np.float32)).astype(bf16)

    # token-major [hi | lo] rows, 512B per token
    xt = np.empty((B, N, 2 * C), dtype=bf16)
    xt[:, :, :C] = hi.transpose(0, 2, 1)
    xt[:, :, C:] = lo.transpose(0, 2, 1)

    hilo = np.stack([hi, lo], axis=1)  # (B, 2, C, N)

    wt = np.ascontiguousarray(
        np.transpose(W.astype(np.float32, copy=False), (1, 2, 0))
    ).reshape(C, K * O).astype(bf16)

    ident = np.eye(128, dtype=bf16)
    negi = (NEG * np.eye(128, dtype=np.float32)).astype(bf16)
    ones128 = np.ones((C, 1), dtype=np.float32)
    ones1 = np.ones((1, 128), dtype=np.float32)

    in_maps = []
    for i in range(NCORES):
        sl = slice(i * BPC, (i + 1) * BPC)
        in_maps.append(
            {
                "x": np.ascontiguousarray(xf[sl]),
                "xt": np.ascontiguousarray(xt[sl]),
                "hilo": np.ascontiguousarray(hilo[sl]),
                "wt": wt,
                "ident": ident,
                "negi": negi,
                "ones128": ones128,
                "ones1": ones1,
            }
        )
    return in_maps


def kernel(x, W):
    global last_results
    from concourse.bass_utils import run_bass_kernel_spmd

    x = np.asarray(x)
    W = np.asarray(W)
    in_maps = _host_prep(x, W)
    nc = _get_program()
    trace = bool(int(os.environ.get("KNN_TRACE", "0")))
    res = run_bass_kernel_spmd(nc, in_maps, list(range(NCORES)), trace=trace)
    last_results = res
    out = np.concatenate([res.results[i]["out"] for i in range(NCORES)], axis=0)
    return out.reshape(B, O, 32, 32).astype(np.float32, copy=False)



# revision 3
# speedup vs baseline: 1.5138x; 1.5138x over previous
"""KNN-conv kernel for Trainium2, data-parallel over batch on 8 NeuronCores.

Problem: for x (32, 128, 32, 32) and conv weight W (128, 128, 9):
  per batch: cosine-sim (1024x1024) over channels, diag -> -1e10, top-8
  neighbors per token (+ self as rank 0), gather neighbor features,
  contract with W.

Strategy per core (4 batches):
  - xn = x * rsqrt(sum x^2) built with Act rsqrt + Pool partition_broadcast
    + Pool multiply, keeping DVE free for the top-k scans
  - sim in true-fp32 PE matmuls (top-k selection needs ~17 mantissa bits;
    bf16/tf32-class dtypes flip ~1-3% of neighbor picks = rel err >> 2e-2)
  - diag forced to -1e10 with an accumulating identity matmul; rank-0
    neighbor is the token itself (k=0 conv term uses self features)
  - top-8: DVE max8 + max_index per 128-row block
  - neighbor gather via dma_gather(transpose=True) from token-major bf16
    rows (256B each); gather positions use a "wrap order" w(m) that swaps
    the block/slot bit-fields of the token id so the index-shuffle DMAs
    are 16B-run / contiguous instead of 2B-element scatters
  - conv = single-pass bf16 matmuls (hi-only features) accumulated in
    fp32 PSUM; output columns are in wrap order, host unpermutes (w is an
    involution)
"""

import os

import numpy as np

B, C, N, K = 32, 128, 1024, 9
O = 128  # out channels
NCORES = 8
BPC = B // NCORES  # batches per core
NEG = -1.0e10

_prog_cache = {}
last_results = None  # BassKernelResults of the most recent run (for test.py)


def _wrap_perm():
    """w(m): swap the high (e) and mid (c) 3-bit fields of m = 128e+16c+q.

    Gather position m holds token w(m); w is an involution.
    """
    m = np.arange(N)
    e, c, q = m >> 7, (m >> 4) & 7, m & 15
    return (c << 7) | (e << 4) | q


def _build_program():
    import concourse.bacc as bacc
    import concourse.mybir as mybir
    from concourse.tile import TileContext

    f32 = mybir.dt.float32
    f32r = mybir.dt.float32r
    bf16 = mybir.dt.bfloat16
    u16 = mybir.dt.uint16
    i16 = mybir.dt.int16
    AF = mybir.ActivationFunctionType

    nc = bacc.Bacc()
    skip = set(os.environ.get("KNN_SKIP", "").split(","))
    sim_f32r = os.environ.get("KNN_SIMDT", "f32") == "f32r"

    x_h = nc.declare_dram_parameter("x", [BPC, C, N], f32, isOutput=False)
    xt_h = nc.declare_dram_parameter("xt", [BPC, N, C], bf16, isOutput=False)
    hiw_h = nc.declare_dram_parameter("hiw", [BPC, C, N], bf16, isOutput=False)
    wt_h = nc.declare_dram_parameter("wt", [C, K * O], bf16, isOutput=False)
    ident_h = nc.declare_dram_parameter("ident", [128, 128], bf16, isOutput=False)
    negi_h = nc.declare_dram_parameter("negi", [128, 128], bf16, isOutput=False)
    ones128_h = nc.declare_dram_parameter("ones128", [C, 1], f32, isOutput=False)
    out_h = nc.declare_dram_parameter("out", [BPC, O, N], f32, isOutput=True)

    idxd_h = nc.dram_tensor("idxd", [BPC, 8192], u16)
    rd_h = nc.dram_tensor("rd", [BPC, N], f32)

    with TileContext(nc) as tc:
        with (
            tc.tile_pool(name="consts", bufs=1) as consts,
            tc.tile_pool(name="xp", bufs=int(os.environ.get("KNN_XP", "2"))) as xp,
            tc.tile_pool(name="sqp", bufs=2) as sqp,
            tc.tile_pool(name="xnp", bufs=int(os.environ.get("KNN_XP", "2"))) as xnp,
            tc.tile_pool(name="rp", bufs=2) as rp,
            tc.tile_pool(name="scp", bufs=int(os.environ.get("KNN_SCP", "3"))) as scp,
            tc.tile_pool(name="smallp", bufs=4) as smallp,
            tc.tile_pool(name="idxp", bufs=2) as idxp,
            tc.tile_pool(name="prp", bufs=int(os.environ.get("KNN_PRP", "4"))) as prp,
            tc.tile_pool(name="hlp", bufs=2) as hlp,
            tc.tile_pool(name="outp", bufs=2) as outp,
            tc.tile_pool(name="psb", bufs=int(os.environ.get("KNN_PSB", "2")), space="PSUM") as psb,
            tc.tile_pool(name="psn", bufs=1, space="PSUM") as psn,
            tc.tile_pool(name="pso", bufs=2, space="PSUM") as pso,
        ):
            wts = consts.tile([C, K * O], bf16, tag="wts")
            nc.sync.dma_start(out=wts[:], in_=wt_h[:])
            ident = consts.tile([128, 128], bf16, tag="ident")
            nc.sync.dma_start(out=ident[:], in_=ident_h[:])
            negi = consts.tile([128, 128], bf16, tag="negi")
            nc.sync.dma_start(out=negi[:], in_=negi_h[:])
            ones128 = consts.tile([C, 1], f32, tag="ones128")
            nc.sync.dma_start(out=ones128[:], in_=ones128_h[:])

            for b in range(BPC):
                # ---- load + normalize -------------------------------------
                X = xp.tile([C, N], f32, tag="x")
                nc.sync.dma_start(out=X[:], in_=x_h[b])

                SQ = sqp.tile([C, N], f32, tag="sq")
                nc.scalar.activation(SQ[:], X[:], AF.Square)

                # norm^2 transposed: n2[p, blk] = sum_c SQ[c, blk*128+p]
                n2 = psn.tile([128, 8], f32, tag="ps_n2")
                for blk in range(8):
                    nc.tensor.matmul(
                        n2[:, blk : blk + 1],
                        SQ[:, blk * 128 : (blk + 1) * 128],
                        ones128[:],
                        start=True,
                        stop=True,
                    )
                sq8 = smallp.tile([128, 8], f32, tag="sq8")
                nc.scalar.activation(sq8[:], n2[:], AF.Sqrt)
                rT = smallp.tile([128, 8], f32, tag="rT")
                nc.vector.reciprocal(rT[:], sq8[:])
                # bounce (128, 8) -> token-ordered (1, 1024) via DRAM
                nc.sync.dma_start(
                    out=rd_h[b].rearrange("(blk p) -> p blk", p=128), in_=rT[:]
                )
                r1 = smallp.tile([1, N], f32, tag="r1")
                nc.sync.dma_start(
                    out=r1[:], in_=rd_h[b].rearrange("(one n) -> one n", one=1)
                )
                # broadcast r over partitions on Pool, multiply on Pool
                R = rp.tile([128, N], f32, tag="r")
                nc.gpsimd.partition_broadcast(R[:], r1[:])
                XN = xnp.tile([C, N], f32, tag="xn")
                nc.gpsimd.tensor_mul(XN[:], X[:], R[:])

                # ---- similarity + top-8 -----------------------------------
                IDX = idxp.tile([128, 64], u16, tag="idx")
                for c in range(8):
                    if "sim" in skip:
                        nc.vector.memset(IDX[:, c : 64 : 8], c)
                        continue
                    ps = psb.tile([128, 1024], f32, tag="ps_big")
                    lhs = XN[:, c * 128 : (c + 1) * 128]
                    rhs = XN[:]
                    if sim_f32r:
                        lhs = lhs.bitcast(f32r)
                        rhs = XN[:].bitcast(f32r)
                    nc.tensor.matmul(
                        ps[:, :512], lhs, rhs[:, :512], start=True, stop=(c >= 4)
                    )
                    nc.tensor.matmul(
                        ps[:, 512:], lhs, rhs[:, 512:], start=True, stop=(c < 4)
                    )
                    # diag block -> -1e10 (accumulate -1e10*I)
                    nc.tensor.matmul(
                        ps[:, c * 128 : c * 128 + 128],
                        ident[:],
                        negi[:],
                        start=False,
                        stop=True,
                    )
                    SC = scp.tile([128, N], f32, tag="sc")
                    nc.scalar.activation(SC[:], ps[:], AF.Copy)
                    if "topk" in skip:
                        nc.vector.memset(IDX[:, c : 64 : 8], c)
                        continue
                    V8 = smallp.tile([128, 8], f32, tag="v8")
                    nc.vector.max(V8[:], SC[:])
                    # rank-major layout: IDX[p, 8j + c] = rank-j idx of
                    # token c*128 + p
                    nc.vector.max_index(IDX[:, c : 64 : 8], V8[:], SC[:])

                # ---- index bounce to wrap-order gather layout -------------
                # want IDXG[16g+q, 64k+8e+c] = IDX[16e+q, 8k+c]
                # DRAM layout: idxd[512q + 64k + 8e + c]; both sides are
                # 3-dim APs with 16B runs (write) / contiguous rows (read)
                idxv = idxd_h[b].rearrange("(q k e c) -> e q k c", q=16, k=8, e=8)
                for e in range(8):
                    nc.sync.dma_start(
                        out=idxv[e],
                        in_=IDX[16 * e : 16 * e + 16, :].rearrange(
                            "q (k c) -> q k c", k=8
                        ),
                    )
                IDXG = idxp.tile([128, 512], u16, tag="idxg")
                for g in range(8):
                    nc.sync.dma_start(
                        out=IDXG[16 * g : 16 * g + 16, :],
                        in_=idxd_h[b].rearrange("(q n) -> q n", q=16),
                    )

                # ---- gathers (channel-major hi via transpose mode) --------
                # gather gi covers ranks 4gi..4gi+3; position within a rank
                # block is m (wrap order), so PR[:, 1024*kl + m] = hi
                # features of the rank-(4gi+kl) neighbor of token w(m)
                prs = []
                for gi in range(2):
                    PR = prp.tile([C, 4 * N], bf16, tag="pr")
                    nc.gpsimd.dma_gather(
                        out_ap=PR[:].rearrange("p (t n) -> p t n", t=1),
                        in_ap=xt_h[b],
                        idxs_ap=IDXG[:, gi * 256 : (gi + 1) * 256].bitcast(i16),
                        num_idxs=4 * N,
                        num_idxs_reg=4 * N,
                        elem_size=C,
                        transpose=True,
                        # single_packet=True overflows the SWDGE packet limit
                        # in transpose mode and crashes the device
                        single_packet=False,
                    )
                    prs.append(PR)
                HIW = hlp.tile([C, N], bf16, tag="hiw")
                nc.sync.dma_start(out=HIW[:], in_=hiw_h[b])

                # ---- conv contraction (bf16 hi-only) ----------------------
                # prime quantized to bf16 adds ~0.3% output error (no top-k
                # amplification downstream), well inside budget
                OUT = outp.tile([O, N], f32, tag="out")
                for h in range(2):
                    PO = pso.tile([O, 512], f32, tag="ps_out")
                    for k in range(1 if "conv" in skip else 9):
                        w_k = wts[:, k * O : (k + 1) * O]
                        if k == 0:
                            src = HIW[:, h * 512 : (h + 1) * 512]
                        else:
                            kl = (k - 1) % 4
                            src = prs[(k - 1) // 4][
                                :, kl * N + h * 512 : kl * N + (h + 1) * 512
                            ]
                        nc.tensor.matmul(
                            PO[:],
                            w_k,
                            src,
                            start=(k == 0),
                            stop=(k == 8 or "conv" in skip),
                        )
                    nc.scalar.activation(OUT[:, h * 512 : (h + 1) * 512], PO[:], AF.Copy)
                nc.sync.dma_start(out=out_h[b], in_=OUT[:])

    nc.compile()
    return nc


def _get_program():
    if "nc" not in _prog_cache:
        _prog_cache["nc"] = _build_program()
    return _prog_cache["nc"]


def _host_prep(x, W):
    """Build per-core input maps from full inputs."""
    import ml_dtypes

    bf16 = ml_dtypes.bfloat16
    xf = np.ascontiguousarray(x.reshape(B, C, N).astype(np.float32, copy=False))
    hi = xf.astype(bf16)
    wp = _wrap_perm()

    # token-major hi rows, 256B per token (natural token order)
    xt = np.ascontiguousarray(hi.transpose(0, 2, 1))
    # self features in wrap order for the k=0 conv term
    hiw = np.ascontiguousarray(hi[:, :, wp])

    wt = np.ascontiguousarray(
        np.transpose(W.astype(np.float32, copy=False), (1, 2, 0))
    ).reshape(C, K * O).astype(bf16)

    ident = np.eye(128, dtype=bf16)
    negi = (NEG * np.eye(128, dtype=np.float32)).astype(bf16)
    ones128 = np.ones((C, 1), dtype=np.float32)

    in_maps = []
    for i in range(NCORES):
        sl = slice(i * BPC, (i + 1) * BPC)
        in_maps.append(
            {
                "x": np.ascontiguousarray(xf[sl]),
                "xt": np.ascontiguousarray(xt[sl]),
                "hiw": np.ascontiguousarray(hiw[sl]),
                "wt": wt,
                "ident": ident,
                "negi": negi,
                "ones128": ones128,
            }
        )
    return in_maps


def kernel(x, W):
    global last_results
    from concourse.bass_utils import run_bass_kernel_spmd

    x = np.asarray(x)
    W = np.asarray(W)
    in_maps = _host_prep(x, W)
    nc = _get_program()
    trace = bool(int(os.environ.get("KNN_TRACE", "0")))
    res = run_bass_kernel_spmd(nc, in_maps, list(range(NCORES)), trace=trace)
    last_results = res
    wp = _wrap_perm()
    out = np.concatenate([res.results[i]["out"] for i in range(NCORES)], axis=0)
    out = out[:, :, wp]  # undo wrap order (w is an involution)
    return out.reshape(B, O, 32, 32).astype(np.float32, copy=False)


# revision 4
# speedup vs baseline: 1.7557x; 1.1598x over previous
"""KNN-conv kernel for Trainium2, data-parallel over batch on 8 NeuronCores.

Problem: for x (32, 128, 32, 32) and conv weight W (128, 128, 9):
  per batch: cosine-sim (1024x1024) over channels, diag -> -1e10, top-8
  neighbors per token (+ self as rank 0), gather neighbor features,
  contract with W.

Strategy per core (4 batches), software-pipelined with emission order
D(b-2), A(b+1), C(b-1), B(b) so no queue head-of-line-blocks the next
batch:
  A: load x, xn = x * 1/sqrt(sum x^2) (Act square/sqrt, DVE recip, Pool
     partition_broadcast + multiply, norms bounced through DRAM to get
     token-major layout)
  B: sim in true-fp32 PE matmuls (top-k selection needs ~17 mantissa
     bits; bf16/tf32-class dtypes flip ~1-3% of picks = rel err >> 2e-2),
     diag forced to -1e10 by an accumulating identity matmul, top-8 via
     DVE max8 + max_index per 128-row block
  C: index shuffle into the dma_gather wrap layout. Gather position m
     holds token w(m), where w swaps the block/slot bit-fields
     (m = 128e+16c+q -> t = 128c+16e+q), which makes the shuffle DMAs
     16B-run / contiguous instead of 2B-element scatters. Two
     dma_gather(transpose=True) calls fetch 4 neighbor ranks each from
     token-major bf16 rows (256B each).
  D: conv = single-pass bf16 matmuls (hi-only features, ~0.3% extra
     error, no top-k amplification) accumulated in fp32 PSUM; output
     columns are in wrap order, host unpermutes (w is an involution).
"""

import os

import numpy as np

B, C, N, K = 32, 128, 1024, 9
O = 128  # out channels
NCORES = 8
BPC = B // NCORES  # batches per core
NEG = -1.0e10

_prog_cache = {}
last_results = None  # BassKernelResults of the most recent run (for test.py)


def _wrap_perm():
    """w(m): swap the high (e) and mid (c) 3-bit fields of m = 128e+16c+q."""
    m = np.arange(N)
    e, c, q = m >> 7, (m >> 4) & 7, m & 15
    return (c << 7) | (e << 4) | q


def _build_program():
    import concourse.bacc as bacc
    import concourse.mybir as mybir
    from concourse.tile import TileContext

    f32 = mybir.dt.float32
    f32r = mybir.dt.float32r
    bf16 = mybir.dt.bfloat16
    u16 = mybir.dt.uint16
    i16 = mybir.dt.int16
    AF = mybir.ActivationFunctionType

    nc = bacc.Bacc()
    skip = set(os.environ.get("KNN_SKIP", "").split(","))
    sim_f32r = os.environ.get("KNN_SIMDT", "f32") == "f32r"

    x_h = nc.declare_dram_parameter("x", [BPC, C, N], f32, isOutput=False)
    xt_h = nc.declare_dram_parameter("xt", [BPC, N, C], bf16, isOutput=False)
    hiw_h = nc.declare_dram_parameter("hiw", [BPC, C, N], bf16, isOutput=False)
    wt_h = nc.declare_dram_parameter("wt", [C, K * O], bf16, isOutput=False)
    ident_h = nc.declare_dram_parameter("ident", [128, 128], bf16, isOutput=False)
    negi_h = nc.declare_dram_parameter("negi", [128, 128], bf16, isOutput=False)
    ones128_h = nc.declare_dram_parameter("ones128", [C, 1], f32, isOutput=False)
    out_h = nc.declare_dram_parameter("out", [BPC, O, N], f32, isOutput=True)

    idxd_h = nc.dram_tensor("idxd", [BPC, 8192], u16)
    rd_h = nc.dram_tensor("rd", [BPC, N], f32)

    with TileContext(nc) as tc:
        with (
            tc.tile_pool(name="consts", bufs=1) as consts,
            tc.tile_pool(name="xp", bufs=3) as xp,
            tc.tile_pool(name="sqp", bufs=2) as sqp,
            tc.tile_pool(name="xnp", bufs=2) as xnp,
            tc.tile_pool(name="rp", bufs=2) as rp,
            tc.tile_pool(name="normp", bufs=4) as normp,
            tc.tile_pool(name="scp", bufs=int(os.environ.get("KNN_SCP", "3"))) as scp,
            tc.tile_pool(name="v8p", bufs=3) as v8p,
            tc.tile_pool(name="idxp", bufs=2) as idxp,
            tc.tile_pool(name="idxgp", bufs=2) as idxgp,
            tc.tile_pool(name="prp", bufs=int(os.environ.get("KNN_PRP", "6"))) as prp,
            tc.tile_pool(name="hlp", bufs=2) as hlp,
            tc.tile_pool(name="outp", bufs=2) as outp,
            tc.tile_pool(name="psb", bufs=int(os.environ.get("KNN_PSB", "2")), space="PSUM") as psb,
            tc.tile_pool(name="psn", bufs=1, space="PSUM") as psn,
            tc.tile_pool(name="pso", bufs=2, space="PSUM") as pso,
        ):
            wts = consts.tile([C, K * O], bf16, tag="wts")
            nc.sync.dma_start(out=wts[:], in_=wt_h[:])
            ident = consts.tile([128, 128], bf16, tag="ident")
            nc.sync.dma_start(out=ident[:], in_=ident_h[:])
            negi = consts.tile([128, 128], bf16, tag="negi")
            nc.sync.dma_start(out=negi[:], in_=negi_h[:])
            ones128 = consts.tile([C, 1], f32, tag="ones128")
            nc.sync.dma_start(out=ones128[:], in_=ones128_h[:])

            st = {}  # per-batch live tiles

            def emit_A(b):
                # load + normalize: xn = x * rsqrt(colsum(x^2))
                X = xp.tile([C, N], f32, tag="x")
                nc.sync.dma_start(out=X[:], in_=x_h[b])
                SQ = sqp.tile([C, N], f32, tag="sq")
                nc.scalar.activation(SQ[:], X[:], AF.Square)
                # norm^2 transposed: n2[p, blk] = sum_c SQ[c, blk*128+p]
                n2 = psn.tile([128, 8], f32, tag="ps_n2")
                for blk in range(8):
                    nc.tensor.matmul(
                        n2[:, blk : blk + 1],
                        SQ[:, blk * 128 : (blk + 1) * 128],
                        ones128[:],
                        start=True,
                        stop=True,
                    )
                sq8 = normp.tile([128, 8], f32, tag="sq8")
                nc.scalar.activation(sq8[:], n2[:], AF.Sqrt)
                rT = normp.tile([128, 8], f32, tag="rT")
                nc.vector.reciprocal(rT[:], sq8[:])
                # bounce (128, 8) -> token-ordered (1, 1024) via DRAM
                nc.sync.dma_start(
                    out=rd_h[b].rearrange("(blk p) -> p blk", p=128), in_=rT[:]
                )
                r1 = normp.tile([1, N], f32, tag="r1")
                nc.sync.dma_start(
                    out=r1[:], in_=rd_h[b].rearrange("(one n) -> one n", one=1)
                )
                # broadcast r over partitions on Pool, multiply on Pool
                R = rp.tile([128, N], f32, tag="r")
                nc.gpsimd.partition_broadcast(R[:], r1[:])
                XN = xnp.tile([C, N], f32, tag="xn")
                nc.gpsimd.tensor_mul(XN[:], X[:], R[:])
                st[b] = {"XN": XN}

            def emit_B(b):
                # similarity + top-8 per 128-row block
                XN = st[b]["XN"]
                IDX = idxp.tile([128, 64], u16, tag="idx")
                for c in range(8):
                    if "sim" in skip:
                        nc.vector.memset(IDX[:, c : 64 : 8], c)
                        continue
                    ps = psb.tile([128, 1024], f32, tag="ps_big")
                    lhs = XN[:, c * 128 : (c + 1) * 128]
                    rhs = XN[:]
                    if sim_f32r:
                        lhs = lhs.bitcast(f32r)
                        rhs = rhs.bitcast(f32r)
                    nc.tensor.matmul(
                        ps[:, :512], lhs, rhs[:, :512], start=True, stop=(c >= 4)
                    )
                    nc.tensor.matmul(
                        ps[:, 512:], lhs, rhs[:, 512:], start=True, stop=(c < 4)
                    )
                    # diag block -> -1e10 (accumulate -1e10*I)
                    nc.tensor.matmul(
                        ps[:, c * 128 : c * 128 + 128],
                        ident[:],
                        negi[:],
                        start=False,
                        stop=True,
                    )
                    SC = scp.tile([128, N], f32, tag="sc")
                    nc.scalar.activation(SC[:], ps[:], AF.Copy)
                    if "topk" in skip:
                        nc.vector.memset(IDX[:, c : 64 : 8], c)
                        continue
                    V8 = v8p.tile([128, 8], f32, tag="v8")
                    nc.vector.max(V8[:], SC[:])
                    # rank-major layout: IDX[p, 8j + c] = rank-j idx of
                    # token c*128 + p
                    nc.vector.max_index(IDX[:, c : 64 : 8], V8[:], SC[:])
                st[b]["IDX"] = IDX

            def emit_C(b):
                # index shuffle into wrap-order gather layout + gathers
                # want IDXG[16g+q, 64k+8e+c] = IDX[16e+q, 8k+c]
                # DRAM layout: idxd[512q + 64k + 8e + c]; both sides are
                # 3-dim APs with 16B runs (write) / contiguous rows (read)
                IDX = st[b]["IDX"]
                idxv = idxd_h[b].rearrange("(q k e c) -> e q k c", q=16, k=8, e=8)
                for e in range(8):
                    nc.sync.dma_start(
                        out=idxv[e],
                        in_=IDX[16 * e : 16 * e + 16, :].rearrange(
                            "q (k c) -> q k c", k=8
                        ),
                    )
                IDXG = idxgp.tile([128, 512], u16, tag="idxg")
                for g in range(8):
                    nc.sync.dma_start(
                        out=IDXG[16 * g : 16 * g + 16, :],
                        in_=idxd_h[b].rearrange("(q n) -> q n", q=16),
                    )
                # gather gi covers ranks 4gi..4gi+3; position within a rank
                # block is m (wrap order), so PR[:, 1024*kl + m] = hi
                # features of the rank-(4gi+kl) neighbor of token w(m)
                prs = []
                for gi in range(2):
                    PR = prp.tile([C, 4 * N], bf16, tag="pr")
                    nc.gpsimd.dma_gather(
                        out_ap=PR[:].rearrange("p (t n) -> p t n", t=1),
                        in_ap=xt_h[b],
                        idxs_ap=IDXG[:, gi * 256 : (gi + 1) * 256].bitcast(i16),
                        num_idxs=4 * N,
                        num_idxs_reg=4 * N,
                        elem_size=C,
                        transpose=True,
                        # single_packet=True overflows the SWDGE packet limit
                        # in transpose mode and crashes the device
                        single_packet=False,
                    )
                    prs.append(PR)
                HIW = hlp.tile([C, N], bf16, tag="hiw")
                nc.sync.dma_start(out=HIW[:], in_=hiw_h[b])
                st[b]["prs"] = prs
                st[b]["HIW"] = HIW

            def emit_D(b):
                # conv contraction (bf16 hi-only) + store
                prs, HIW = st[b]["prs"], st[b]["HIW"]
                OUT = outp.tile([O, N], f32, tag="out")
                for h in range(2):
                    PO = pso.tile([O, 512], f32, tag="ps_out")
                    for k in range(1 if "conv" in skip else 9):
                        w_k = wts[:, k * O : (k + 1) * O]
                        if k == 0:
                            src = HIW[:, h * 512 : (h + 1) * 512]
                        else:
                            kl = (k - 1) % 4
                            src = prs[(k - 1) // 4][
                                :, kl * N + h * 512 : kl * N + (h + 1) * 512
                            ]
                        nc.tensor.matmul(
                            PO[:],
                            w_k,
                            src,
                            start=(k == 0),
                            stop=(k == 8 or "conv" in skip),
                        )
                    nc.scalar.activation(
                        OUT[:, h * 512 : (h + 1) * 512], PO[:], AF.Copy
                    )
                # store from the Act queue so it never head-of-line-blocks
                # the SP queue's next-batch loads
                nc.scalar.dma_start(out=out_h[b], in_=OUT[:])
                del st[b]

            emit_A(0)
            for b in range(BPC):
                if b >= 2:
                    emit_D(b - 2)
                if b + 1 < BPC:
                    emit_A(b + 1)
                if b >= 1:
                    emit_C(b - 1)
                emit_B(b)
            emit_C(BPC - 1)
            emit_D(BPC - 2)
            emit_D(BPC - 1)

    nc.compile()
    return nc


def _get_program():
    if "nc" not in _prog_cache:
        _prog_cache["nc"] = _build_program()
    return _prog_cache["nc"]


def _host_prep(x, W):
    """Build per-core input maps from full inputs."""
    import ml_dtypes

    bf16 = ml_dtypes.bfloat16
    xf = np.ascontiguousarray(x.reshape(B, C, N).astype(np.float32, copy=False))
    hi = xf.astype(bf16)
    wp = _wrap_perm()

    # token-major hi rows, 256B per token (natural token order)
    xt = np.ascontiguousarray(hi.transpose(0, 2, 1))
    # self features in wrap order for the k=0 conv term
    hiw = np.ascontiguousarray(hi[:, :, wp])

    wt = np.ascontiguousarray(
        np.transpose(W.astype(np.float32, copy=False), (1, 2, 0))
    ).reshape(C, K * O).astype(bf16)

    ident = np.eye(128, dtype=bf16)
    negi = (NEG * np.eye(128, dtype=np.float32)).astype(bf16)
    ones128 = np.ones((C, 1), dtype=np.float32)

    in_maps = []
    for i in range(NCORES):
        sl = slice(i * BPC, (i + 1) * BPC)
        in_maps.append(
            {
                "x": np.ascontiguousarray(xf[sl]),
                "xt": np.ascontiguousarray(xt[sl]),
                "hiw": np.ascontiguousarray(hiw[sl]),
                "wt": wt,
                "ident": ident,
                "negi": negi,
                "ones128": ones128,
            }
        )
    return in_maps


def kernel(x, W):
    global last_results
    from concourse.bass_utils import run_bass_kernel_spmd

    x = np.asarray(x)
    W = np.asarray(W)
    in_maps = _host_prep(x, W)
    nc = _get_program()
    trace = bool(int(os.environ.get("KNN_TRACE", "0")))
    res = run_bass_kernel_spmd(nc, in_maps, list(range(NCORES)), trace=trace)
    last_results = res
    wp = _wrap_perm()
    out = np.concatenate([res.results[i]["out"] for i in range(NCORES)], axis=0)
    out = out[:, :, wp]  # undo wrap order (w is an involution)
    return out.reshape(B, O, 32, 32).astype(np.float32, copy=False)
